# revision 1
# baseline (speedup 1.0000x reference)
"""CIEDE2000 ColorLoss kernel for Trainium2, 8 NeuronCores, data-parallel.

Full inputs x, y: [32, 3, 512, 512] f32 NCHW in [0, 1].
Output: scalar f32 = mean(ciede2000(rgb2lab(x), rgb2lab(y))) / 100.

Sharding: batch dim split 4 images per core (8 cores). Each core computes a
per-partition sum of deltaE over its 4*512*512 pixels; host combines.

Math notes (vs the jax reference):
  - clip(x,0,1) dropped: inputs are uniform [0,1).
  - a = 500*(fx-fy), b = 200*(fy-fz) carried unscaled (alpha, beta); all
    constants folded: C = 100*sqrt((5a)^2+(2b)^2), (25/50)^7 = 1/128 etc.
  - hue handled in principal range (-pi, pi]: h = 2*arctan(b/(C'+a'))
    (half-angle atan2), hbar via circular-mean with predicated wrap.
  - dHp via dHp^2 = dEp^2 - dCp^2 (exact identity), sign via cross product.
  - all sqrts as exp(0.5*ln) to stay in one ACT table set; sin range-reduced
    into (-pi,pi] with add_range_wrap (Sin LUT is only valid there).
  - sin(2*dtheta) via odd polynomial (arg in [0, pi/3]).
"""
import os
import sys

sys.path.insert(0, "/opt/trn_rl_repo")

import numpy as np
import concourse.bacc as bacc
import concourse.tile as tile
import concourse.mybir as mybir
from concourse.bass_utils import run_bass_kernel_spmd
from contextlib import ExitStack

F32 = mybir.dt.float32
I32 = mybir.dt.int32
AF = mybir.ActivationFunctionType
ALU = mybir.AluOpType

P = 128          # partitions
FCH = 1024       # chunk free dim
NCHUNK = 8       # chunks per core: P*FCH*NCHUNK = 1048576 px = 4 imgs
NCORE = 8
IMGS_PER_CORE = 4
ROWS_PER_IMG = 32  # partitions per image: 262144 / 8192

PI = float(np.pi)
LNP = float(np.log(1.0 / 128.0))     # ln((25/50)^7)
# sRGB -> XYZ rows divided by D65 white
_M = np.array([[0.412453, 0.357580, 0.180423],
               [0.212671, 0.715160, 0.072169],
               [0.019334, 0.119193, 0.950227]], dtype=np.float64)
_W = np.array([0.95047, 1.0, 1.08883], dtype=np.float64)
MW = (_M / _W[:, None]).astype(np.float32)  # [3,3], row k = xyz_k coeffs

B_LIN = float(0.055 / 1.055)
K1 = float(PI / 3)
K3 = float(-(PI / 3) ** 3 / 6.0)
K5 = float((PI / 3) ** 5 / 120.0)
GSCALE = float(180.0 / (25.0 * PI))
GBIAS = 3.4

_BIASES = [B_LIN, LNP, -66.0, 20.0, GBIAS]

_NC_CACHE = {}


def _reg_consts(nc, values):
    for v in values:
        v = float(v)
        if (F32, v) not in nc.const_aps.aps:
            t = nc.alloc_sbuf_tensor(f"constf32_{repr(v)}", [128, 1], F32)
            nc.gpsimd.memset(t.ap(), v)
            nc.const_aps.aps[(F32, v)] = t.ap()
    nc.all_engine_barrier()


def build_nc():
    nc = bacc.Bacc("TRN2", target_bir_lowering=False, debug=False)
    _reg_consts(nc, _BIASES)
    A = nc.scalar
    V = nc.vector
    Gp = nc.gpsimd

    # inputs viewed as [img, ch, row, chunk, col]
    shp = [IMGS_PER_CORE, 3, ROWS_PER_IMG, NCHUNK, FCH]
    x_d = nc.dram_tensor("x", shp, F32, kind="ExternalInput").ap()
    y_d = nc.dram_tensor("y", shp, F32, kind="ExternalInput").ap()
    out_d = nc.dram_tensor("out", [P, 1], F32, kind="ExternalOutput").ap()

    with tile.TileContext(nc) as tc, ExitStack() as ctx:
        pool = ctx.enter_context(tc.tile_pool(name="main", bufs=1))
        inpool = ctx.enter_context(tc.tile_pool(name="in", bufs=1))

        NTMP = 14
        tmp_i = [0]

        def T(tag):
            """Long-lived named plane."""
            return pool.tile([P, FCH], F32, tag=tag, name=tag)

        def tmp():
            """Short-lived temp from a rotating tag set."""
            tag = f"tmp{tmp_i[0] % NTMP}"
            tmp_i[0] += 1
            return pool.tile([P, FCH], F32, tag=tag, name=tag)

        acc = pool.tile([P, NCHUNK], F32, tag="acc", name="acc")

        for k in range(NCHUNK):
            # ---- load 6 channel planes ----
            planes = {}
            for img, src in ((1, x_d), (2, y_d)):
                for c in range(3):
                    t = inpool.tile([P, FCH], F32, tag=f"in{img}{c}",
                                    name=f"in{img}{c}")
                    # partition pi = im*32 + r  <->  src[im, c, r, k, :]
                    for im in range(IMGS_PER_CORE):
                        nc.sync.dma_start(
                            t[im * ROWS_PER_IMG:(im + 1) * ROWS_PER_IMG, :],
                            src[im, c, :, k, :],
                        )
                    planes[(img, c)] = t

            # ---- stage 1: rgb -> (alpha, beta, fy) per image (lnexp set) ----
            fy = {}
            alpha = {}
            beta = {}
            for img in (1, 2):
                lin = []
                for c in range(3):
                    src = planes[(img, c)]
                    t1 = tmp()
                    A.activation(t1[:], src[:], AF.Ln,
                                 scale=float(1 / 1.055), bias=B_LIN)
                    u = tmp()
                    A.activation(u[:], t1[:], AF.Exp, scale=2.4)
                    m = tmp()
                    Gp.tensor_scalar(m[:], src[:], 0.04045, None, ALU.is_gt)
                    lc = pool.tile([P, FCH], F32, tag=f"lin{c}", name=f"lin{c}")
                    V.tensor_scalar(lc[:], src[:], float(1 / 12.92), None,
                                    ALU.mult)
                    V.copy_predicated(lc[:], m[:].bitcast(I32), u[:])
                    lin.append(lc)
                f = []
                for kk in range(3):
                    mk = MW[kk]
                    t0 = tmp()
                    V.tensor_scalar(t0[:], lin[0][:], float(mk[0]), None,
                                    ALU.mult)
                    t1 = tmp()
                    V.scalar_tensor_tensor(t1[:], lin[1][:], float(mk[1]),
                                           t0[:], ALU.mult, ALU.add)
                    xk = tmp()
                    V.scalar_tensor_tensor(xk[:], lin[2][:], float(mk[2]),
                                           t1[:], ALU.mult, ALU.add)
                    lf = tmp()
                    A.activation(lf[:], xk[:], AF.Ln)
                    uf = tmp()
                    A.activation(uf[:], lf[:], AF.Exp, scale=float(1 / 3))
                    mf = tmp()
                    Gp.tensor_scalar(mf[:], xk[:], 0.008856, None, ALU.is_gt)
                    if kk == 1:
                        fk = pool.tile([P, FCH], F32, tag=f"fy{img}",
                                       name=f"fy{img}")
                    else:
                        fk = pool.tile([P, FCH], F32, tag=f"f{kk}",
                                       name=f"f{kk}")
                    V.tensor_scalar(fk[:], xk[:], 7.787, 0.13793103,
                                    ALU.mult, ALU.add)
                    V.copy_predicated(fk[:], mf[:].bitcast(I32), uf[:])
                    f.append(fk)
                al = T(f"alpha{img}")
                V.tensor_sub(al[:], f[0][:], f[1][:])
                be = T(f"beta{img}")
                V.tensor_sub(be[:], f[1][:], f[2][:])
                alpha[img], beta[img], fy[img] = al, be, f[1]

            # ---- L chain early (Square/lnexp ok in this set) ----
            fysum = tmp()
            Gp.tensor_tensor(fysum[:], fy[1][:], fy[2][:], ALU.add)
            dfy = T("dfy")
            Gp.tensor_tensor(dfy[:], fy[2][:], fy[1][:], ALU.subtract)
            L50 = tmp()
            A.activation(L50[:], fysum[:], AF.Square, scale=58.0, bias=-66.0)
            lld = tmp()
            A.activation(lld[:], L50[:], AF.Ln, bias=20.0)
            rLd = tmp()
            A.activation(rLd[:], lld[:], AF.Exp, scale=-0.5)
            uL = tmp()
            V.tensor_mul(uL[:], L50[:], rLd[:])
            SL = T("SL")
            V.tensor_scalar(SL[:], uL[:], 0.015, 1.0, ALU.mult, ALU.add)

            # ---- stage 2: chroma chains (lnexp set) ----
            qb = {}
            Cc = {}
            for img in (1, 2):
                qa = tmp()
                A.activation(qa[:], alpha[img][:], AF.Square, scale=5.0)
                qbt = T(f"qb{img}")
                A.activation(qbt[:], beta[img][:], AF.Square, scale=2.0)
                qb[img] = qbt
                s = tmp()
                V.tensor_add(s[:], qa[:], qbt[:])
                l = tmp()
                A.activation(l[:], s[:], AF.Ln)
                Ct = tmp()
                A.activation(Ct[:], l[:], AF.Exp, scale=0.5)
                Cc[img] = Ct
            Sc = tmp()
            Gp.tensor_tensor(Sc[:], Cc[1][:], Cc[2][:], ALU.add)
            lc = tmp()
            A.activation(lc[:], Sc[:], AF.Ln)
            e1 = tmp()
            A.activation(e1[:], lc[:], AF.Exp, scale=-7.0, bias=LNP)
            t1g = tmp()
            A.activation(t1g[:], e1[:], AF.Ln, bias=1.0)
            rsqG = tmp()
            A.activation(rsqG[:], t1g[:], AF.Exp, scale=-0.5)
            g1 = tmp()
            V.tensor_scalar(g1[:], rsqG[:], -0.5, 1.5, ALU.mult, ALU.add)

            ap = {}
            Cp = {}
            for img in (1, 2):
                apt = T(f"ap{img}")
                V.tensor_mul(apt[:], g1[:], alpha[img][:])
                ap[img] = apt
                qap = tmp()
                A.activation(qap[:], apt[:], AF.Square, scale=5.0)
                sp = tmp()
                V.tensor_add(sp[:], qap[:], qb[img][:])
                lp = tmp()
                A.activation(lp[:], sp[:], AF.Ln)
                Cpt = T(f"Cp{img}")
                A.activation(Cpt[:], lp[:], AF.Exp, scale=0.5)
                Cp[img] = Cpt
            dCp = T("dCp")
            V.tensor_sub(dCp[:], Cp[2][:], Cp[1][:])
            Scp = T("Scp")
            Gp.tensor_tensor(Scp[:], Cp[1][:], Cp[2][:], ALU.add)
            SCt = T("SCt")
            V.tensor_scalar(SCt[:], Scp[:], 2.25, 1.0, ALU.mult, ALU.add)
            lcp = tmp()
            A.activation(lcp[:], Scp[:], AF.Ln)
            e2 = tmp()
            A.activation(e2[:], lcp[:], AF.Exp, scale=-7.0, bias=LNP)
            t2g = tmp()
            A.activation(t2g[:], e2[:], AF.Ln, bias=1.0)
            rsqC = T("rsqC")
            A.activation(rsqC[:], t2g[:], AF.Exp, scale=-0.5)

            dap = tmp()
            V.tensor_sub(dap[:], ap[2][:], ap[1][:])
            dbe = tmp()
            Gp.tensor_tensor(dbe[:], beta[2][:], beta[1][:], ALU.subtract)
            qda = tmp()
            A.activation(qda[:], dap[:], AF.Square, scale=5.0)
            qdb = tmp()
            A.activation(qdb[:], dbe[:], AF.Square, scale=2.0)
            dE2 = tmp()
            V.tensor_add(dE2[:], qda[:], qdb[:])
            qdc = tmp()
            A.activation(qdc[:], dCp[:], AF.Square)
            diff = tmp()
            V.tensor_sub(diff[:], dE2[:], qdc[:])
            difr = tmp()
            A.activation(difr[:], diff[:], AF.Relu)
            ldf = tmp()
            A.activation(ldf[:], difr[:], AF.Ln)
            sqd = T("sqd")
            A.activation(sqd[:], ldf[:], AF.Exp, scale=0.5)

            cr0 = tmp()
            Gp.tensor_tensor(cr0[:], beta[2][:], alpha[1][:], ALU.mult)
            cr1 = tmp()
            Gp.tensor_tensor(cr1[:], beta[1][:], alpha[2][:], ALU.mult)
            cr = tmp()
            V.tensor_sub(cr[:], cr0[:], cr1[:])
            sgn = T("sgn")
            A.activation(sgn[:], cr[:], AF.Sign)

            qq = {}
            for img in (1, 2):
                den = tmp()
                V.scalar_tensor_tensor(den[:], ap[img][:], 5.0, Cp[img][:],
                                       ALU.mult, ALU.add)
                dc = tmp()
                V.tensor_scalar(dc[:], den[:], 1e-30, None, ALU.max)
                r = tmp()
                V.reciprocal_approx_fast(r[:], dc[:])
                qt = T(f"q{img}")
                V.scalar_tensor_tensor(qt[:], beta[img][:], 2.0, r[:],
                                       ALU.mult, ALU.mult)
                qq[img] = qt

            # ---- stage 3: hue (trig set) ----
            at = {}
            for img in (1, 2):
                att = tmp()
                A.activation(att[:], qq[img][:], AF.Arctan)
                at[img] = att
            raw = tmp()
            Gp.tensor_tensor(raw[:], at[2][:], at[1][:], ALU.subtract)
            mwq = tmp()
            A.activation(mwq[:], raw[:], AF.Square)
            mw = tmp()
            V.tensor_scalar(mw[:], mwq[:], float(PI * PI / 4), None, ALU.is_gt)
            hb = T("hb")
            V.tensor_add(hb[:], at[1][:], at[2][:])
            alt = tmp()
            V.add_range_wrap(alt[:], hb[:], PI, PI, 2 * PI)
            V.copy_predicated(hb[:], mw[:].bitcast(I32), alt[:])

            w2 = tmp()
            V.tensor_scalar(w2[:], hb[:], 2.0, None, ALU.mult)
            a1w = tmp()
            V.add_range_wrap(a1w[:], hb[:], float(PI / 3), PI, 2 * PI)
            c1t = tmp()
            A.activation(c1t[:], a1w[:], AF.Sin)
            a2w = tmp()
            V.add_range_wrap(a2w[:], w2[:], float(PI / 2), PI, 2 * PI)
            c2t = tmp()
            A.activation(c2t[:], a2w[:], AF.Sin)
            hb2p = tmp()
            V.add_range_wrap(hb2p[:], w2[:], 0.0, PI, 2 * PI)
            tmp3 = tmp()
            Gp.tensor_tensor(tmp3[:], hb2p[:], hb[:], ALU.add)
            a3w = tmp()
            V.add_range_wrap(a3w[:], tmp3[:], float(PI / 30 + PI / 2), PI, 2 * PI)
            c3t = tmp()
            A.activation(c3t[:], a3w[:], AF.Sin)
            w4 = tmp()
            V.tensor_scalar(w4[:], hb2p[:], 2.0, None, ALU.mult)
            a4w = tmp()
            V.add_range_wrap(a4w[:], w4[:], 0.4712389, PI, 2 * PI)
            c4t = tmp()
            A.activation(c4t[:], a4w[:], AF.Sin)

            Tt = tmp()
            V.tensor_scalar(Tt[:], c1t[:], -0.17, 1.0, ALU.mult, ALU.add)
            Tt2 = tmp()
            V.scalar_tensor_tensor(Tt2[:], c2t[:], 0.24, Tt[:], ALU.mult, ALU.add)
            Tt3 = tmp()
            V.scalar_tensor_tensor(Tt3[:], c3t[:], 0.32, Tt2[:], ALU.mult, ALU.add)
            Tt4 = tmp()
            V.scalar_tensor_tensor(Tt4[:], c4t[:], -0.20, Tt3[:], ALU.mult, ALU.add)
            qg = tmp()
            A.activation(qg[:], hb[:], AF.Square, scale=GSCALE, bias=GBIAS)

            # ---- stage 4: assemble (lnexp set) ----
            eg = tmp()
            A.activation(eg[:], qg[:], AF.Exp, scale=-1.0)
            wg = tmp()
            A.activation(wg[:], eg[:], AF.Square)
            pp = tmp()
            V.tensor_scalar(pp[:], wg[:], K5, K3, ALU.mult, ALU.add)
            p2 = tmp()
            V.tensor_mul(p2[:], wg[:], pp[:])
            s2d = tmp()
            V.scalar_tensor_tensor(s2d[:], p2[:], K1, eg[:], ALU.add, ALU.mult)
            RTp = T("RTp")
            V.tensor_mul(RTp[:], s2d[:], rsqC[:])

            rC = tmp()
            V.reciprocal_approx_fast(rC[:], SCt[:])
            tC = tmp()
            V.tensor_mul(tC[:], dCp[:], rC[:])
            uh = tmp()
            Gp.tensor_tensor(uh[:], Scp[:], Tt4[:], ALU.mult)
            SH = tmp()
            V.tensor_scalar(SH[:], uh[:], 0.75, 1.0, ALU.mult, ALU.add)
            rH = tmp()
            V.reciprocal_approx_fast(rH[:], SH[:])
            tH = tmp()
            V.tensor_mul(tH[:], sqd[:], rH[:])
            rL = tmp()
            V.reciprocal_approx_fast(rL[:], SL[:])
            tL = tmp()
            V.tensor_mul(tL[:], dfy[:], rL[:])

            zL = tmp()
            A.activation(zL[:], tL[:], AF.Square, scale=116.0)
            zC = tmp()
            A.activation(zC[:], tC[:], AF.Square, scale=100.0)
            zH = tmp()
            A.activation(zH[:], tH[:], AF.Square, scale=100.0)
            w2t = tmp()
            Gp.tensor_tensor(w2t[:], tC[:], tH[:], ALU.mult)
            ct1 = tmp()
            Gp.tensor_tensor(ct1[:], w2t[:], sgn[:], ALU.mult)
            w2f = tmp()
            V.scalar_tensor_tensor(w2f[:], RTp[:], -20000.0, ct1[:],
                                   ALU.mult, ALU.mult)
            F1 = tmp()
            Gp.tensor_tensor(F1[:], zL[:], zC[:], ALU.add)
            F2 = tmp()
            Gp.tensor_tensor(F2[:], F1[:], zH[:], ALU.add)
            F3 = tmp()
            V.tensor_add(F3[:], F2[:], w2f[:])
            Fr = tmp()
            A.activation(Fr[:], F3[:], AF.Relu)
            lF = tmp()
            A.activation(lF[:], Fr[:], AF.Ln)
            dE = tmp()
            A.activation(dE[:], lF[:], AF.Exp, scale=0.5,
                         accum_out=acc[:, k:k + 1])

        # final: reduce acc cols -> [P,1], DMA out
        accsum = pool.tile([P, 1], F32, tag="accsum", name="accsum")
        V.tensor_reduce(accsum[:], acc[:], mybir.AxisListType.X, ALU.add)
        nc.sync.dma_start(out_d[:], accsum[:])

    nc.compile()
    return nc


def _get_nc():
    if "nc" not in _NC_CACHE:
        _NC_CACHE["nc"] = build_nc()
    return _NC_CACHE["nc"]


def kernel(x: np.ndarray, y: np.ndarray) -> np.ndarray:
    assert x.shape == (32, 3, 512, 512) and y.shape == (32, 3, 512, 512)
    nc = _get_nc()
    shp = (IMGS_PER_CORE, 3, ROWS_PER_IMG, NCHUNK, FCH)
    xs = np.ascontiguousarray(x, dtype=np.float32)
    ys = np.ascontiguousarray(y, dtype=np.float32)
    in_maps = []
    for c in range(NCORE):
        xi = xs[c * IMGS_PER_CORE:(c + 1) * IMGS_PER_CORE].reshape(shp)
        yi = ys[c * IMGS_PER_CORE:(c + 1) * IMGS_PER_CORE].reshape(shp)
        in_maps.append({"x": xi, "y": yi})
    trace = bool(int(os.environ.get("COLOR_TRACE", "0")))
    res = run_bass_kernel_spmd(nc, in_maps, core_ids=list(range(NCORE)),
                               trace=trace)
    _NC_CACHE["last_results"] = res
    total = np.float64(0.0)
    for c in range(NCORE):
        total += np.float64(res.results[c]["out"].sum())
    npix = 32 * 512 * 512
    return np.float32(total / npix / 100.0)



# revision 10
# speedup vs baseline: 2.9901x; 2.9901x over previous
"""CIEDE2000 ColorLoss kernel for Trainium2, 8 NeuronCores, data-parallel.

Full inputs x, y: [32, 3, 512, 512] f32 NCHW in [0, 1].
Output: scalar f32 = mean(ciede2000(rgb2lab(x), rgb2lab(y))) / 100.

Sharding: batch dim split 4 images per core (8 cores). Each core computes a
per-partition sum of deltaE over its 4*512*512 pixels; host combines.

Design (v2):
  - Scalar (ACT) engine does ONLY Ln/Exp -> single table set, zero
    ACT_TABLE_LOAD switches after warmup.
  - Zero GpSimd compute (it contends with the DVE on the shared SBUF port).
  - Hue handled algebraically: unit bisector (cos hbar, sin hbar) via
    vector addition u1*C2' + u2*C1'; T weighting as parity-split
    polynomial P(c) + s*Q(c); dtheta gaussian approximated as
    exp(10.5*(cos(hbar-275deg)-1)); dHp = 2000*cross*sqrt(C1'C2')/|v|
    (exact identity, no trig, sign included).
  - Fused custom DVE ops (selects, sum-of-squares, lincombs, polys) with
    immediate constants; bf16 planes for 2x stock-DVE throughput.
  - Math error vs reference ~5e-4 (numpy-simulated, bf16 rounding incl).
"""
import os
import sys

sys.path.insert(0, "/opt/trn_rl_repo")

import numpy as np
import concourse.bacc as bacc
import concourse.tile as tile
import concourse.mybir as mybir
import concourse.dve_ops as D
from concourse.dve_spec import (
    Spec, Src0, Src1, C0, C1, C2, relu, sq, select, maxx, minn,
    lower as dve_lower, _has_src1,
)
from concourse.dve_uop import DveOpSpec
from concourse.bass_utils import run_bass_kernel_spmd
from contextlib import ExitStack

F32 = mybir.dt.float32
BF16 = mybir.dt.bfloat16
AF = mybir.ActivationFunctionType
ALU = mybir.AluOpType

P = 128          # partitions
FCH = 1024       # chunk free dim
NCHUNK = 8       # chunks per core: P*FCH*NCHUNK = 1048576 px = 4 imgs
NCORE = 8
IMGS_PER_CORE = 4
ROWS_PER_IMG = 32  # partitions per image: 262144 / 8192

# ---- constants -------------------------------------------------------------
PI = float(np.pi)
LNP = float(np.log(1.0 / 128.0))     # ln((25/50)^7)
B_LIN = float(0.055 / 1.055)
K1 = float(PI / 3)
K3 = float(-(PI / 3) ** 3 / 6.0)
K5 = float((PI / 3) ** 5 / 120.0)

_M = np.array([[0.412453, 0.357580, 0.180423],
               [0.212671, 0.715160, 0.072169],
               [0.019334, 0.119193, 0.950227]], dtype=np.float64)
_W = np.array([0.95047, 1.0, 1.08883], dtype=np.float64)
MW = (_M / _W[:, None]).astype(np.float32)  # [3,3]

_D2R = PI / 180.0


def _cd(d):
    return float(np.cos(d * _D2R))


def _sd(d):
    return float(np.sin(d * _D2R))


# T(h) = P(cos h) + sin(h) * Q(cos h), parity-split Chebyshev coefficients
TA0 = 1.0 - 0.24 - 0.20 * _cd(63)
TA1 = -0.17 * _cd(30) - 0.96 * _cd(6)
TA2 = 0.48 + 1.6 * _cd(63)
TA3 = 1.28 * _cd(6)
TA4 = -1.6 * _cd(63)
TQ0 = -0.17 * _sd(30) + 0.32 * _sd(6)
TQ1 = 0.80 * _sd(63)
TQ2 = -1.28 * _sd(6)
TQ3 = -1.60 * _sd(63)
C275 = _cd(275)
S275 = _sd(275)

_BIASES = [0.0, B_LIN, LNP, 1.0, 20.0, 1e-12, -10.5, 1e-30]

_NC_CACHE = {}


# ---- custom DVE ops --------------------------------------------------------
def _register_op(name, spec, subdim=False):
    if name in D._SUB_OPCODE_FOR_NAME:
        return next(o for o in D.OPS if o.name == name)
    row = 1 + len(D.OPS)
    assert row < 0x20, "custom DVE opcode rows exhausted"
    D._SUB_OPCODE_FOR_NAME[name] = row
    shas = {}
    for ver in ("v3",):
        s = DveOpSpec(name=name, opcode=row, uops=dve_lower(spec, ver=ver),
                      rd1_en=_has_src1(spec))
        shas[ver] = s.sha(ver)
    op = D.DveOp(name, spec, subdim, shas)
    D.OPS.append(op)
    D.CUSTOM_DVE_SPECS[name] = spec
    return op


# select(x > c0, a, x*c1 + c2) : sRGB + cbrt branch
OP_SEL_GT = _register_op("ANT_SEL_GT", Spec(
    body=select(Src0 > C0, Src1, Src0 * C1 + C2)))
# sq(a*c0) + sq(b*c1) : chroma^2, |v|^2, dE^2 partials
OP_SUMSQ = _register_op("ANT_SUMSQ", Spec(
    body=sq(Src0 * C0) + sq(Src1 * C1)))
# a*c0 + b*c1 + c2
OP_LINCOMB = _register_op("ANT_LINCOMB", Spec(
    body=Src0 * C0 + Src1 * C1 + C2))
# sq((a+b)*c0 + c1) : L50^2
OP_ADD_AFF_SQ = _register_op("ANT_ADD_AFF_SQ", Spec(
    body=sq((Src0 + Src1) * C0 + C1)))
# (a*c0)*b + c1 : SL, SH
OP_MULMUL_ADD = _register_op("ANT_MULMUL_ADD", Spec(
    body=(Src0 * C0) * Src1 + C1))
# sq(a*c0)*sq(b) : zL
OP_SQSQ_MUL = _register_op("ANT_SQSQ_MUL", Spec(
    body=sq(Src0 * C0) * sq(Src1)))
# (a*c0 + c1)*b : (1+G)*alpha, T parity pieces
OP_AFF_MUL = _register_op("ANT_AFF_MUL", Spec(
    body=(Src0 * C0 + C1) * Src1))
# sq(a)*c0 + a*c1 + c2 : quad poly
OP_QUAD = _register_op("ANT_QUAD", Spec(
    body=sq(Src0) * C0 + Src0 * C1 + C2))
# (sq(a)*b + c0)*a : sin odd-poly tail
OP_SIN_POLY = _register_op("ANT_SIN_POLY", Spec(
    body=(sq(Src0) * Src1 + C0) * Src0))
# relu(a + b) : final F clamp
OP_ADD_RELU = _register_op("ANT_ADD_RELU", Spec(
    body=relu(Src0 + Src1)))
# clamp(a*c0*b, c1, c2) : unit-vector components (degenerate-hue guard)
OP_SMUL_CLAMP = _register_op("ANT_SMUL_CLAMP", Spec(
    body=minn(maxx(Src0 * C0 * Src1, C1), C2)))
# clamp(a, c1, c2)*b*c0 : dHp = 200*g12*clamp(sin(dh/2))
OP_CLAMP_MUL = _register_op("ANT_CLAMP_MUL", Spec(
    body=minn(maxx(Src0, C1), C2) * Src1 * C0))


def _reg_consts(nc, values):
    for v in values:
        v = float(v)
        if (F32, v) not in nc.const_aps.aps:
            t = nc.alloc_sbuf_tensor(f"constf32_{repr(v)}", [128, 1], F32)
            nc.gpsimd.memset(t.ap(), v)
            nc.const_aps.aps[(F32, v)] = t.ap()
    nc.all_engine_barrier()


def build_nc(debug_dump=False):
    nc = bacc.Bacc("TRN2", target_bir_lowering=False, debug=False)
    _reg_consts(nc, _BIASES)
    A = nc.scalar
    V = nc.vector

    # inputs viewed as [img, ch, row, chunk, col]
    shp = [IMGS_PER_CORE, 3, ROWS_PER_IMG, NCHUNK, FCH]
    x_d = nc.dram_tensor("x", shp, F32, kind="ExternalInput").ap()
    y_d = nc.dram_tensor("y", shp, F32, kind="ExternalInput").ap()
    out_d = nc.dram_tensor("out", [P, 1], F32, kind="ExternalOutput").ap()

    dbg_tiles = {}

    def dbg(name, t):
        if debug_dump:
            dbg_tiles[name] = (t, t.shape[1], t.dtype)

    W = FCH          # single-plane width
    W2 = 2 * FCH     # pair width
    W6 = 6 * FCH

    with tile.TileContext(nc) as tc, ExitStack() as ctx:
        inpool = ctx.enter_context(tc.tile_pool(name="in", bufs=1))
        pool = ctx.enter_context(tc.tile_pool(name="main", bufs=1))

        NTMP = 10
        NTMPF = 5
        tmp_i = [0, 0]

        def T_(tag, w=W, dt=BF16):
            return pool.tile([P, w], dt, tag=tag, name=tag)

        def tmp(dt=BF16):
            if dt is F32:
                tag = f"ftmp{tmp_i[1] % NTMPF}"
                tmp_i[1] += 1
            else:
                tag = f"tmp{tmp_i[0] % NTMP}"
                tmp_i[0] += 1
            return pool.tile([P, W], dt, tag=tag, name=tag)

        acc = pool.tile([P, NCHUNK], F32, tag="acc", name="acc")

        def S(t, i, n=1):
            """Free-dim slice covering planes [i, i+n) of a multi-plane tile."""
            return t[:, i * FCH:(i + n) * FCH]

        for k in range(NCHUNK):
            # ---- load 6 channel planes: [xr yr xg yg xb yb] ----
            IN = inpool.tile([P, W6], F32, tag="in6", name="in6")
            for c in range(3):
                for img, src in ((0, x_d), (1, y_d)):
                    pl = 2 * c + img
                    for im in range(IMGS_PER_CORE):
                        nc.sync.dma_start(
                            IN[im * ROWS_PER_IMG:(im + 1) * ROWS_PER_IMG,
                               pl * FCH:(pl + 1) * FCH],
                            src[im, c, :, k, :],
                        )

            # ---- front-end: srgb -> lin -> xyz -> f (cbrt branch) ----
            W1 = T_("w1", W6)   # LNU -> GAM -> LIN
            A.activation(W1[:], IN[:], AF.Ln, scale=float(1 / 1.055),
                         bias=B_LIN)
            A.activation(W1[:], W1[:], AF.Exp, scale=2.4)
            V._custom_dve(OP_SEL_GT, out=W1[:], in0=IN[:], in1=W1[:],
                          s0=0.04045, s1=float(1 / 12.92), imm2=0.0)

            XYZ = T_("xyz", W6)
            for kk in range(3):
                mk = MW[kk]
                t0 = pool.tile([P, W2], BF16, tag=f"xt{kk % 2}",
                               name=f"xt{kk % 2}")
                V.tensor_scalar(t0[:], S(W1, 0, 2), float(mk[0]), None,
                                ALU.mult)
                V.scalar_tensor_tensor(t0[:], S(W1, 2, 2), float(mk[1]),
                                       t0[:], ALU.mult, ALU.add)
                V.scalar_tensor_tensor(S(XYZ, 2 * kk, 2), S(W1, 4, 2),
                                       float(mk[2]), t0[:], ALU.mult, ALU.add)

            WF = T_("wf", W6)   # LNX -> CBR -> F
            A.activation(WF[:], XYZ[:], AF.Ln)
            A.activation(WF[:], WF[:], AF.Exp, scale=float(1 / 3))
            V._custom_dve(OP_SEL_GT, out=WF[:], in0=XYZ[:], in1=WF[:],
                          s0=0.008856, s1=7.787, imm2=0.13793103)

            AL = T_("al", W2)
            BE = T_("be", W2)
            V.tensor_sub(AL[:], S(WF, 0, 2), S(WF, 2, 2))
            V.tensor_sub(BE[:], S(WF, 2, 2), S(WF, 4, 2))

            # ---- L chain ----
            L50 = T_("l50")
            V._custom_dve(OP_ADD_AFF_SQ, out=L50[:], in0=S(WF, 2),
                          in1=S(WF, 3), s0=58.0, s1=-66.0)
            lld = tmp()
            A.activation(lld[:], L50[:], AF.Ln, bias=20.0)
            rLd = tmp()
            A.activation(rLd[:], lld[:], AF.Exp, scale=-0.5)
            SLf = T_("slf", W, F32)
            V._custom_dve(OP_MULMUL_ADD, out=SLf[:], in0=L50[:], in1=rLd[:],
                          s0=0.015, s1=1.0)
            rL = T_("rl", W, F32)
            V.reciprocal_approx_fast(rL[:], SLf[:])
            dfy = T_("dfy")
            V.tensor_sub(dfy[:], S(WF, 3), S(WF, 2))
            zL = T_("zl")
            V._custom_dve(OP_SQSQ_MUL, out=zL[:], in0=dfy[:], in1=rL[:],
                          s0=116.0)

            # ---- chroma + G ----
            C2p = T_("c2p", W2)
            V._custom_dve(OP_SUMSQ, out=C2p[:], in0=AL[:], in1=BE[:],
                          s0=5.0, s1=2.0)
            A.activation(C2p[:], C2p[:], AF.Ln)
            CCp = C2p
            A.activation(CCp[:], CCp[:], AF.Exp, scale=0.5)
            Sc = tmp()
            V.tensor_add(Sc[:], S(CCp, 0), S(CCp, 1))
            lcS = tmp()
            A.activation(lcS[:], Sc[:], AF.Ln)
            e1 = tmp()
            A.activation(e1[:], lcS[:], AF.Exp, scale=-7.0, bias=LNP)
            l1g = tmp()
            A.activation(l1g[:], e1[:], AF.Ln, bias=1.0)
            rsqG = tmp()
            A.activation(rsqG[:], l1g[:], AF.Exp, scale=-0.5)
            AP = T_("ap", W2)
            V._custom_dve(OP_AFF_MUL, out=S(AP, 0), in0=rsqG[:],
                          in1=S(AL, 0), s0=-0.5, s1=1.5)
            V._custom_dve(OP_AFF_MUL, out=S(AP, 1), in0=rsqG[:],
                          in1=S(AL, 1), s0=-0.5, s1=1.5)

            CP2 = T_("cp2", W2)
            V._custom_dve(OP_SUMSQ, out=CP2[:], in0=AP[:], in1=BE[:],
                          s0=5.0, s1=2.0)
            LP = CP2
            A.activation(LP[:], CP2[:], AF.Ln)
            # CPS = [Cpy | Cpx]  (swapped -> pair products via one TT mult)
            CPS = T_("cps", W2)
            A.activation(S(CPS, 0), S(LP, 1), AF.Exp, scale=0.5)
            A.activation(S(CPS, 1), S(LP, 0), AF.Exp, scale=0.5)
            dCp = T_("dcp")
            V.tensor_sub(dCp[:], S(CPS, 0), S(CPS, 1))
            Scp = T_("scp")
            V.tensor_add(Scp[:], S(CPS, 0), S(CPS, 1))
            # SCH = [SC | SH] fp32 for the paired reciprocal
            SCH = T_("sch", W2, F32)
            V.tensor_scalar(S(SCH, 0), Scp[:], 2.25, 1.0, ALU.mult, ALU.add)
            lcp = tmp()
            A.activation(lcp[:], Scp[:], AF.Ln)
            e2 = tmp()
            A.activation(e2[:], lcp[:], AF.Exp, scale=-7.0, bias=LNP)
            l2g = tmp()
            A.activation(l2g[:], e2[:], AF.Ln, bias=1.0)
            rsqC = T_("rsqc")
            A.activation(rsqC[:], l2g[:], AF.Exp, scale=-0.5)

            # ---- hue: cross, sqrt(C1C2), bisector (cb, sb) ----
            # fp32 chain: bf16*bf16 products are exact in fp32, keeping the
            # dHp identity exact; clamps guard the near-antipodal tail.
            m1 = tmp(F32)
            V.tensor_mul(m1[:], S(AP, 0), S(BE, 1))
            m2 = tmp(F32)
            V.tensor_mul(m2[:], S(AP, 1), S(BE, 0))
            cross = T_("cross", W, F32)
            V.tensor_sub(cross[:], m1[:], m2[:])
            lsum = tmp()
            V.tensor_add(lsum[:], S(LP, 0), S(LP, 1))
            g12 = tmp()
            A.activation(g12[:], lsum[:], AF.Exp, scale=0.25)
            prA = pool.tile([P, W2], F32, tag="pra", name="pra")
            V.tensor_mul(prA[:], AP[:], CPS[:])
            vx = tmp(F32)
            V.tensor_add(vx[:], S(prA, 0), S(prA, 1))
            prB = pool.tile([P, W2], F32, tag="pra", name="prb")
            V.tensor_mul(prB[:], BE[:], CPS[:])
            vy = tmp(F32)
            V.tensor_add(vy[:], S(prB, 0), S(prB, 1))
            n2 = tmp(F32)
            V._custom_dve(OP_SUMSQ, out=n2[:], in0=vx[:], in1=vy[:],
                          s0=5.0, s1=2.0)
            ln2 = tmp(F32)
            A.activation(ln2[:], n2[:], AF.Ln, bias=1e-12)
            rn = tmp(F32)
            A.activation(rn[:], ln2[:], AF.Exp, scale=-0.5)
            cb = T_("cb")
            V._custom_dve(OP_SMUL_CLAMP, out=cb[:], in0=vx[:], in1=rn[:],
                          s0=5.0, s1=-1.0, imm2=1.0)
            sb = T_("sb")
            V._custom_dve(OP_SMUL_CLAMP, out=sb[:], in0=vy[:], in1=rn[:],
                          s0=2.0, s1=-1.0, imm2=1.0)
            sh2 = tmp(F32)
            V.scalar_tensor_tensor(sh2[:], cross[:], 10.0, rn[:], ALU.mult,
                                   ALU.mult)
            dHps = T_("dhps", W, F32)
            V._custom_dve(OP_CLAMP_MUL, out=dHps[:], in0=sh2[:], in1=g12[:],
                          s0=200.0, s1=-1.0, imm2=1.0)

            # ---- T = P(cb) + sb*Q(cb) ----
            u = T_("u")
            V.tensor_mul(u[:], cb[:], cb[:])
            cs = tmp()
            V.tensor_mul(cs[:], cb[:], sb[:])
            Pe = tmp()
            V._custom_dve(OP_QUAD, out=Pe[:], in0=u[:], s0=TA4, s1=TA2,
                          imm2=TA0)
            Po = tmp()
            V._custom_dve(OP_AFF_MUL, out=Po[:], in0=u[:], in1=cb[:],
                          s0=TA3, s1=TA1)
            Qe = tmp()
            V._custom_dve(OP_AFF_MUL, out=Qe[:], in0=u[:], in1=sb[:],
                          s0=TQ2, s1=TQ0)
            Qo = tmp()
            V._custom_dve(OP_AFF_MUL, out=Qo[:], in0=u[:], in1=cs[:],
                          s0=TQ3, s1=TQ1)
            t10 = tmp()
            V.tensor_add(t10[:], Pe[:], Po[:])
            t11 = tmp()
            V.tensor_add(t11[:], Qe[:], Qo[:])
            Tt = tmp()
            V.tensor_add(Tt[:], t10[:], t11[:])
            V._custom_dve(OP_MULMUL_ADD, out=S(SCH, 1), in0=Scp[:],
                          in1=Tt[:], s0=0.75, s1=1.0)
            RR = T_("rr", W2, F32)
            V.reciprocal_approx_fast(RR[:], SCH[:])
            tC = T_("tc")
            V.tensor_mul(tC[:], dCp[:], S(RR, 0))
            tH = T_("th", W, F32)
            V.tensor_mul(tH[:], dHps[:], S(RR, 1))

            # ---- RT gaussian + sin poly ----
            c275 = tmp()
            V._custom_dve(OP_LINCOMB, out=c275[:], in0=cb[:], in1=sb[:],
                          s0=C275, s1=S275, imm2=0.0)
            eg = tmp()
            A.activation(eg[:], c275[:], AF.Exp, scale=10.5, bias=-10.5)
            P1 = tmp()
            V._custom_dve(OP_QUAD, out=P1[:], in0=eg[:], s0=K5, s1=0.0,
                          imm2=K3)
            s2d = tmp()
            V._custom_dve(OP_SIN_POLY, out=s2d[:], in0=eg[:], in1=P1[:],
                          s0=K1)
            w1 = tmp()
            V.tensor_mul(w1[:], s2d[:], rsqC[:])

            # ---- F assembly + dE ----
            q1 = tmp()
            V._custom_dve(OP_SUMSQ, out=q1[:], in0=tC[:], in1=tH[:],
                          s0=100.0, s1=1.0)
            q2 = tmp()
            V.scalar_tensor_tensor(q2[:], tC[:], -200.0, tH[:], ALU.mult,
                                   ALU.mult)
            q3 = tmp()
            V.tensor_mul(q3[:], q2[:], w1[:])
            F1 = tmp()
            V.tensor_add(F1[:], q1[:], q3[:])
            Ff = tmp()
            V._custom_dve(OP_ADD_RELU, out=Ff[:], in0=F1[:], in1=zL[:])
            lF = tmp()
            A.activation(lF[:], Ff[:], AF.Ln, bias=1e-30)
            dE = tmp()
            A.activation(dE[:], lF[:], AF.Exp, scale=0.5,
                         accum_out=acc[:, k:k + 1])

            if debug_dump and k == 0:
                for nm, t in [("lin", W1), ("f", WF), ("al", AL), ("be", BE),
                              ("zl", zL), ("ccp", CCp), ("rsqg", rsqG),
                              ("ap", AP), ("cps", CPS), ("dcp", dCp),
                              ("scp", Scp), ("rsqc", rsqC), ("cross", cross),
                              ("cb", cb), ("sb", sb), ("dhps", dHps),
                              ("tt", Tt), ("sch", SCH), ("tc", tC),
                              ("th", tH), ("eg", eg), ("s2d", s2d),
                              ("ff", Ff), ("de", dE)]:
                    w = t.shape[-1]
                    dd = nc.dram_tensor(f"dbg_{nm}", [P, w], t.dtype,
                                        kind="ExternalOutput").ap()
                    nc.sync.dma_start(dd[:], t[:])

        # final: reduce acc cols -> [P,1], DMA out
        accsum = pool.tile([P, 1], F32, tag="accsum", name="accsum")
        V.tensor_reduce(accsum[:], acc[:], mybir.AxisListType.X, ALU.add)
        nc.sync.dma_start(out_d[:], accsum[:])

    nc.compile()
    return nc


def _get_nc():
    if "nc" not in _NC_CACHE:
        _NC_CACHE["nc"] = build_nc()
    return _NC_CACHE["nc"]


def kernel(x: np.ndarray, y: np.ndarray) -> np.ndarray:
    assert x.shape == (32, 3, 512, 512) and y.shape == (32, 3, 512, 512)
    nc = _get_nc()
    shp = (IMGS_PER_CORE, 3, ROWS_PER_IMG, NCHUNK, FCH)
    xs = np.ascontiguousarray(x, dtype=np.float32)
    ys = np.ascontiguousarray(y, dtype=np.float32)
    in_maps = []
    for c in range(NCORE):
        xi = xs[c * IMGS_PER_CORE:(c + 1) * IMGS_PER_CORE].reshape(shp)
        yi = ys[c * IMGS_PER_CORE:(c + 1) * IMGS_PER_CORE].reshape(shp)
        in_maps.append({"x": xi, "y": yi})
    trace = bool(int(os.environ.get("COLOR_TRACE", "0")))
    res = run_bass_kernel_spmd(nc, in_maps, core_ids=list(range(NCORE)),
                               trace=trace)
    _NC_CACHE["last_results"] = res
    total = np.float64(0.0)
    for c in range(NCORE):
        total += np.float64(res.results[c]["out"].sum())
    npix = 32 * 512 * 512
    return np.float32(total / npix / 100.0)


# revision 12
# speedup vs baseline: 3.3422x; 1.1178x over previous
"""CIEDE2000 ColorLoss kernel for Trainium2, 8 NeuronCores, data-parallel.

Full inputs x, y: [32, 3, 512, 512] f32 NCHW in [0, 1].
Output: scalar f32 = mean(ciede2000(rgb2lab(x), rgb2lab(y))) / 100.

Sharding: batch dim split 4 images per core (8 cores). Each core computes a
per-partition sum of deltaE over its 4*512*512 pixels; host combines.

Design (v2):
  - Scalar (ACT) engine does ONLY Ln/Exp -> single table set, zero
    ACT_TABLE_LOAD switches after warmup.
  - Zero GpSimd compute (it contends with the DVE on the shared SBUF port).
  - Hue handled algebraically: unit bisector (cos hbar, sin hbar) via
    vector addition u1*C2' + u2*C1'; T weighting as parity-split
    polynomial P(c) + s*Q(c); dtheta gaussian approximated as
    exp(10.5*(cos(hbar-275deg)-1)); dHp = 2000*cross*sqrt(C1'C2')/|v|
    (exact identity, no trig, sign included).
  - Fused custom DVE ops (selects, sum-of-squares, lincombs, polys) with
    immediate constants; bf16 planes for 2x stock-DVE throughput.
  - Math error vs reference ~5e-4 (numpy-simulated, bf16 rounding incl).
"""
import os
import sys

sys.path.insert(0, "/opt/trn_rl_repo")

import numpy as np
import concourse.bacc as bacc
import concourse.tile as tile
import concourse.mybir as mybir
import concourse.dve_ops as D
from concourse.dve_spec import (
    Spec, Src0, Src1, C0, C1, C2, relu, sq, select, maxx, minn,
    lower as dve_lower, _has_src1,
)
from concourse.dve_uop import DveOpSpec
from concourse.bass_utils import run_bass_kernel_spmd
from contextlib import ExitStack

F32 = mybir.dt.float32
BF16 = mybir.dt.bfloat16
AF = mybir.ActivationFunctionType
ALU = mybir.AluOpType

P = 128          # partitions
FCH = 1024       # chunk free dim
NCHUNK = 8       # chunks per core: P*FCH*NCHUNK = 1048576 px = 4 imgs
NCORE = 8
IMGS_PER_CORE = 4
ROWS_PER_IMG = 32  # partitions per image: 262144 / 8192

# ---- constants -------------------------------------------------------------
PI = float(np.pi)
LNP = float(np.log(1.0 / 128.0))     # ln((25/50)^7)
B_LIN = float(0.055 / 1.055)
K1 = float(PI / 3)
K3 = float(-(PI / 3) ** 3 / 6.0)
K5 = float((PI / 3) ** 5 / 120.0)

_M = np.array([[0.412453, 0.357580, 0.180423],
               [0.212671, 0.715160, 0.072169],
               [0.019334, 0.119193, 0.950227]], dtype=np.float64)
_W = np.array([0.95047, 1.0, 1.08883], dtype=np.float64)
MW = (_M / _W[:, None]).astype(np.float32)  # [3,3]

_D2R = PI / 180.0


def _cd(d):
    return float(np.cos(d * _D2R))


def _sd(d):
    return float(np.sin(d * _D2R))


# T(h) = P(cos h) + sin(h) * Q(cos h), parity-split Chebyshev coefficients
TA0 = 1.0 - 0.24 - 0.20 * _cd(63)
TA1 = -0.17 * _cd(30) - 0.96 * _cd(6)
TA2 = 0.48 + 1.6 * _cd(63)
TA3 = 1.28 * _cd(6)
TA4 = -1.6 * _cd(63)
TQ0 = -0.17 * _sd(30) + 0.32 * _sd(6)
TQ1 = 0.80 * _sd(63)
TQ2 = -1.28 * _sd(6)
TQ3 = -1.60 * _sd(63)
C275 = _cd(275)
S275 = _sd(275)

_BIASES = [0.0, B_LIN, LNP, 1.0, 20.0, 1e-12, -10.5, 1e-30]

_NC_CACHE = {}


# ---- custom DVE ops --------------------------------------------------------
def _register_op(name, spec, subdim=False):
    if name in D._SUB_OPCODE_FOR_NAME:
        return next(o for o in D.OPS if o.name == name)
    row = 1 + len(D.OPS)
    assert row < 0x20, "custom DVE opcode rows exhausted"
    D._SUB_OPCODE_FOR_NAME[name] = row
    shas = {}
    for ver in ("v3",):
        s = DveOpSpec(name=name, opcode=row, uops=dve_lower(spec, ver=ver),
                      rd1_en=_has_src1(spec))
        shas[ver] = s.sha(ver)
    op = D.DveOp(name, spec, subdim, shas)
    D.OPS.append(op)
    D.CUSTOM_DVE_SPECS[name] = spec
    return op


# select(x > c0, a, x*c1 + c2) : sRGB + cbrt branch
OP_SEL_GT = _register_op("ANT_SEL_GT", Spec(
    body=select(Src0 > C0, Src1, Src0 * C1 + C2)))
# sq(a*c0) + sq(b*c1) : chroma^2, |v|^2, dE^2 partials
OP_SUMSQ = _register_op("ANT_SUMSQ", Spec(
    body=sq(Src0 * C0) + sq(Src1 * C1)))
# a*c0 + b*c1 + c2
OP_LINCOMB = _register_op("ANT_LINCOMB", Spec(
    body=Src0 * C0 + Src1 * C1 + C2))
# sq((a+b)*c0 + c1) : L50^2
OP_ADD_AFF_SQ = _register_op("ANT_ADD_AFF_SQ", Spec(
    body=sq((Src0 + Src1) * C0 + C1)))
# (a*c0)*b + c1 : SL, SH
OP_MULMUL_ADD = _register_op("ANT_MULMUL_ADD", Spec(
    body=(Src0 * C0) * Src1 + C1))
# sq(a*c0)*sq(b) : zL
OP_SQSQ_MUL = _register_op("ANT_SQSQ_MUL", Spec(
    body=sq(Src0 * C0) * sq(Src1)))
# (a*c0 + c1)*b : (1+G)*alpha, T parity pieces
OP_AFF_MUL = _register_op("ANT_AFF_MUL", Spec(
    body=(Src0 * C0 + C1) * Src1))
# sq(a)*c0 + a*c1 + c2 : quad poly
OP_QUAD = _register_op("ANT_QUAD", Spec(
    body=sq(Src0) * C0 + Src0 * C1 + C2))
# (sq(a)*b + c0)*a : sin odd-poly tail
OP_SIN_POLY = _register_op("ANT_SIN_POLY", Spec(
    body=(sq(Src0) * Src1 + C0) * Src0))
# relu(a + b) : final F clamp
OP_ADD_RELU = _register_op("ANT_ADD_RELU", Spec(
    body=relu(Src0 + Src1)))
# clamp(a*c0*b, c1, c2) : unit-vector components (degenerate-hue guard)
OP_SMUL_CLAMP = _register_op("ANT_SMUL_CLAMP", Spec(
    body=minn(maxx(Src0 * C0 * Src1, C1), C2)))
# clamp(a, c1, c2)*b*c0 : dHp = 200*g12*clamp(sin(dh/2))
OP_CLAMP_MUL = _register_op("ANT_CLAMP_MUL", Spec(
    body=minn(maxx(Src0, C1), C2) * Src1 * C0))


# Force Ln and Exp to resolve to the combined natural_log_exp set: the
# greedy table-load pass otherwise alternates natural_log <-> exp_and_others
# on every Ln/Exp switch (~2.7us per ACT_TABLE_LOAD, ~20 per chunk).
_ORIG_GAT = None


def _install_lnexp_table_patch():
    global _ORIG_GAT
    if _ORIG_GAT is not None:
        return
    import concourse.hw_specs as hw_specs
    _ORIG_GAT = hw_specs.get_activation_tables

    def _gat(arch):
        t = _ORIG_GAT(arch)
        out = {}
        for name, fns in t.items():
            if name != "natural_log_exp_and_others":
                fns = {f for f in fns if f not in (AF.Ln, AF.Exp)}
            out[name] = fns
        return out

    hw_specs.get_activation_tables = _gat
    bacc.get_activation_tables = _gat


def _reg_consts(nc, values):
    for v in values:
        v = float(v)
        if (F32, v) not in nc.const_aps.aps:
            t = nc.alloc_sbuf_tensor(f"constf32_{repr(v)}", [128, 1], F32)
            nc.gpsimd.memset(t.ap(), v)
            nc.const_aps.aps[(F32, v)] = t.ap()
    nc.all_engine_barrier()


def build_nc(debug_dump=False):
    _install_lnexp_table_patch()
    nc = bacc.Bacc("TRN2", target_bir_lowering=False, debug=False)
    _reg_consts(nc, _BIASES)
    A = nc.scalar
    V = nc.vector

    # inputs viewed as [img, ch, row, chunk, col]
    shp = [IMGS_PER_CORE, 3, ROWS_PER_IMG, NCHUNK, FCH]
    x_d = nc.dram_tensor("x", shp, F32, kind="ExternalInput").ap()
    y_d = nc.dram_tensor("y", shp, F32, kind="ExternalInput").ap()
    out_d = nc.dram_tensor("out", [P, 1], F32, kind="ExternalOutput").ap()

    dbg_tiles = {}

    def dbg(name, t):
        if debug_dump:
            dbg_tiles[name] = (t, t.shape[1], t.dtype)

    W = FCH          # single-plane width
    W2 = 2 * FCH     # pair width
    W6 = 6 * FCH

    with tile.TileContext(nc) as tc, ExitStack() as ctx:
        inpool = ctx.enter_context(tc.tile_pool(name="in", bufs=1))
        pool = ctx.enter_context(tc.tile_pool(name="main", bufs=1))

        NTMP = 10
        NTMPF = 5
        tmp_i = [0, 0]

        def T_(tag, w=W, dt=BF16):
            return pool.tile([P, w], dt, tag=tag, name=tag)

        def tmp(dt=BF16):
            if dt is F32:
                tag = f"ftmp{tmp_i[1] % NTMPF}"
                tmp_i[1] += 1
            else:
                tag = f"tmp{tmp_i[0] % NTMP}"
                tmp_i[0] += 1
            return pool.tile([P, W], dt, tag=tag, name=tag)

        acc = pool.tile([P, NCHUNK], F32, tag="acc", name="acc")

        def S(t, i, n=1):
            """Free-dim slice covering planes [i, i+n) of a multi-plane tile."""
            return t[:, i * FCH:(i + n) * FCH]

        for k in range(NCHUNK):
            # ---- load 6 channel planes: [xr yr xg yg xb yb] ----
            IN = inpool.tile([P, W6], F32, tag="in6", name="in6")
            for c in range(3):
                for img, src in ((0, x_d), (1, y_d)):
                    pl = 2 * c + img
                    for im in range(IMGS_PER_CORE):
                        nc.sync.dma_start(
                            IN[im * ROWS_PER_IMG:(im + 1) * ROWS_PER_IMG,
                               pl * FCH:(pl + 1) * FCH],
                            src[im, c, :, k, :],
                        )

            # ---- front-end: srgb -> lin -> xyz -> f (cbrt branch) ----
            W1 = T_("w1", W6)   # LNU -> GAM -> LIN
            A.activation(W1[:], IN[:], AF.Ln, scale=float(1 / 1.055),
                         bias=B_LIN)
            A.activation(W1[:], W1[:], AF.Exp, scale=2.4)
            V._custom_dve(OP_SEL_GT, out=W1[:], in0=IN[:], in1=W1[:],
                          s0=0.04045, s1=float(1 / 12.92), imm2=0.0)

            XYZ = T_("xyz", W6)
            for kk in range(3):
                mk = MW[kk]
                t0 = pool.tile([P, W2], BF16, tag=f"xt{kk % 2}",
                               name=f"xt{kk % 2}")
                V.tensor_scalar(t0[:], S(W1, 0, 2), float(mk[0]), None,
                                ALU.mult)
                V.scalar_tensor_tensor(t0[:], S(W1, 2, 2), float(mk[1]),
                                       t0[:], ALU.mult, ALU.add)
                V.scalar_tensor_tensor(S(XYZ, 2 * kk, 2), S(W1, 4, 2),
                                       float(mk[2]), t0[:], ALU.mult, ALU.add)

            WF = T_("wf", W6)   # LNX -> CBR -> F
            A.activation(WF[:], XYZ[:], AF.Ln)
            A.activation(WF[:], WF[:], AF.Exp, scale=float(1 / 3))
            V._custom_dve(OP_SEL_GT, out=WF[:], in0=XYZ[:], in1=WF[:],
                          s0=0.008856, s1=7.787, imm2=0.13793103)

            AL = T_("al", W2)
            BE = T_("be", W2)
            V.tensor_sub(AL[:], S(WF, 0, 2), S(WF, 2, 2))
            V.tensor_sub(BE[:], S(WF, 2, 2), S(WF, 4, 2))

            # ---- L chain ----
            L50 = T_("l50")
            V._custom_dve(OP_ADD_AFF_SQ, out=L50[:], in0=S(WF, 2),
                          in1=S(WF, 3), s0=58.0, s1=-66.0)
            lld = tmp()
            A.activation(lld[:], L50[:], AF.Ln, bias=20.0)
            rLd = tmp()
            A.activation(rLd[:], lld[:], AF.Exp, scale=-0.5)
            SLf = T_("slf", W, F32)
            V._custom_dve(OP_MULMUL_ADD, out=SLf[:], in0=L50[:], in1=rLd[:],
                          s0=0.015, s1=1.0)
            rL = T_("rl", W, F32)
            V.reciprocal_approx_fast(rL[:], SLf[:])
            dfy = T_("dfy")
            V.tensor_sub(dfy[:], S(WF, 3), S(WF, 2))
            zL = T_("zl")
            V._custom_dve(OP_SQSQ_MUL, out=zL[:], in0=dfy[:], in1=rL[:],
                          s0=116.0)

            # ---- chroma + G ----
            C2p = T_("c2p", W2)
            V._custom_dve(OP_SUMSQ, out=C2p[:], in0=AL[:], in1=BE[:],
                          s0=5.0, s1=2.0)
            A.activation(C2p[:], C2p[:], AF.Ln)
            CCp = C2p
            A.activation(CCp[:], CCp[:], AF.Exp, scale=0.5)
            Sc = tmp()
            V.tensor_add(Sc[:], S(CCp, 0), S(CCp, 1))
            lcS = tmp()
            A.activation(lcS[:], Sc[:], AF.Ln)
            e1 = tmp()
            A.activation(e1[:], lcS[:], AF.Exp, scale=-7.0, bias=LNP)
            l1g = tmp()
            A.activation(l1g[:], e1[:], AF.Ln, bias=1.0)
            rsqG = tmp()
            A.activation(rsqG[:], l1g[:], AF.Exp, scale=-0.5)
            AP = T_("ap", W2)
            V._custom_dve(OP_AFF_MUL, out=S(AP, 0), in0=rsqG[:],
                          in1=S(AL, 0), s0=-0.5, s1=1.5)
            V._custom_dve(OP_AFF_MUL, out=S(AP, 1), in0=rsqG[:],
                          in1=S(AL, 1), s0=-0.5, s1=1.5)

            CP2 = T_("cp2", W2)
            V._custom_dve(OP_SUMSQ, out=CP2[:], in0=AP[:], in1=BE[:],
                          s0=5.0, s1=2.0)
            LP = CP2
            A.activation(LP[:], CP2[:], AF.Ln)
            # CPS = [Cpy | Cpx]  (swapped -> pair products via one TT mult)
            CPS = T_("cps", W2)
            A.activation(S(CPS, 0), S(LP, 1), AF.Exp, scale=0.5)
            A.activation(S(CPS, 1), S(LP, 0), AF.Exp, scale=0.5)
            dCp = T_("dcp")
            V.tensor_sub(dCp[:], S(CPS, 0), S(CPS, 1))
            Scp = T_("scp")
            V.tensor_add(Scp[:], S(CPS, 0), S(CPS, 1))
            # SCH = [SC | SH] fp32 for the paired reciprocal
            SCH = T_("sch", W2, F32)
            V.tensor_scalar(S(SCH, 0), Scp[:], 2.25, 1.0, ALU.mult, ALU.add)
            lcp = tmp()
            A.activation(lcp[:], Scp[:], AF.Ln)
            e2 = tmp()
            A.activation(e2[:], lcp[:], AF.Exp, scale=-7.0, bias=LNP)
            l2g = tmp()
            A.activation(l2g[:], e2[:], AF.Ln, bias=1.0)
            rsqC = T_("rsqc")
            A.activation(rsqC[:], l2g[:], AF.Exp, scale=-0.5)

            # ---- hue: cross, sqrt(C1C2), bisector (cb, sb) ----
            # fp32 chain: bf16*bf16 products are exact in fp32, keeping the
            # dHp identity exact; clamps guard the near-antipodal tail.
            m1 = tmp(F32)
            V.tensor_mul(m1[:], S(AP, 0), S(BE, 1))
            m2 = tmp(F32)
            V.tensor_mul(m2[:], S(AP, 1), S(BE, 0))
            cross = T_("cross", W, F32)
            V.tensor_sub(cross[:], m1[:], m2[:])
            lsum = tmp()
            V.tensor_add(lsum[:], S(LP, 0), S(LP, 1))
            g12 = tmp()
            A.activation(g12[:], lsum[:], AF.Exp, scale=0.25)
            prA = pool.tile([P, W2], BF16, tag="pra", name="pra")
            V.tensor_mul(prA[:], AP[:], CPS[:])
            vx = tmp(F32)
            V.tensor_add(vx[:], S(prA, 0), S(prA, 1))
            prB = pool.tile([P, W2], BF16, tag="pra", name="prb")
            V.tensor_mul(prB[:], BE[:], CPS[:])
            vy = tmp(F32)
            V.tensor_add(vy[:], S(prB, 0), S(prB, 1))
            n2 = tmp(F32)
            V._custom_dve(OP_SUMSQ, out=n2[:], in0=vx[:], in1=vy[:],
                          s0=5.0, s1=2.0)
            ln2 = tmp(F32)
            A.activation(ln2[:], n2[:], AF.Ln, bias=1e-12)
            rn = tmp(F32)
            A.activation(rn[:], ln2[:], AF.Exp, scale=-0.5)
            cb = T_("cb")
            V._custom_dve(OP_SMUL_CLAMP, out=cb[:], in0=vx[:], in1=rn[:],
                          s0=5.0, s1=-1.0, imm2=1.0)
            sb = T_("sb")
            V._custom_dve(OP_SMUL_CLAMP, out=sb[:], in0=vy[:], in1=rn[:],
                          s0=2.0, s1=-1.0, imm2=1.0)
            sh2 = tmp(F32)
            V.scalar_tensor_tensor(sh2[:], cross[:], 10.0, rn[:], ALU.mult,
                                   ALU.mult)
            dHps = T_("dhps", W, F32)
            V._custom_dve(OP_CLAMP_MUL, out=dHps[:], in0=sh2[:], in1=g12[:],
                          s0=200.0, s1=-1.0, imm2=1.0)

            # ---- T = P(cb) + sb*Q(cb) ----
            u = T_("u")
            V.tensor_mul(u[:], cb[:], cb[:])
            cs = tmp()
            V.tensor_mul(cs[:], cb[:], sb[:])
            Pe = tmp()
            V._custom_dve(OP_QUAD, out=Pe[:], in0=u[:], s0=TA4, s1=TA2,
                          imm2=TA0)
            Po = tmp()
            V._custom_dve(OP_AFF_MUL, out=Po[:], in0=u[:], in1=cb[:],
                          s0=TA3, s1=TA1)
            Qe = tmp()
            V._custom_dve(OP_AFF_MUL, out=Qe[:], in0=u[:], in1=sb[:],
                          s0=TQ2, s1=TQ0)
            Qo = tmp()
            V._custom_dve(OP_AFF_MUL, out=Qo[:], in0=u[:], in1=cs[:],
                          s0=TQ3, s1=TQ1)
            t10 = tmp()
            V.tensor_add(t10[:], Pe[:], Po[:])
            t11 = tmp()
            V.tensor_add(t11[:], Qe[:], Qo[:])
            Tt = tmp()
            V.tensor_add(Tt[:], t10[:], t11[:])
            V._custom_dve(OP_MULMUL_ADD, out=S(SCH, 1), in0=Scp[:],
                          in1=Tt[:], s0=0.75, s1=1.0)
            RR = SCH
            V.reciprocal_approx_fast(RR[:], SCH[:])
            tC = T_("tc")
            V.tensor_mul(tC[:], dCp[:], S(RR, 0))
            tH = T_("th", W, F32)
            V.tensor_mul(tH[:], dHps[:], S(RR, 1))

            # ---- RT gaussian + sin poly ----
            c275 = tmp()
            V._custom_dve(OP_LINCOMB, out=c275[:], in0=cb[:], in1=sb[:],
                          s0=C275, s1=S275, imm2=0.0)
            eg = tmp()
            A.activation(eg[:], c275[:], AF.Exp, scale=10.5, bias=-10.5)
            P1 = tmp()
            V._custom_dve(OP_QUAD, out=P1[:], in0=eg[:], s0=K5, s1=0.0,
                          imm2=K3)
            s2d = tmp()
            V._custom_dve(OP_SIN_POLY, out=s2d[:], in0=eg[:], in1=P1[:],
                          s0=K1)
            w1 = tmp()
            V.tensor_mul(w1[:], s2d[:], rsqC[:])

            # ---- F assembly + dE ----
            q1 = tmp()
            V._custom_dve(OP_SUMSQ, out=q1[:], in0=tC[:], in1=tH[:],
                          s0=100.0, s1=1.0)
            q2 = tmp()
            V.scalar_tensor_tensor(q2[:], tC[:], -200.0, tH[:], ALU.mult,
                                   ALU.mult)
            q3 = tmp()
            V.tensor_mul(q3[:], q2[:], w1[:])
            F1 = tmp()
            V.tensor_add(F1[:], q1[:], q3[:])
            Ff = tmp()
            V._custom_dve(OP_ADD_RELU, out=Ff[:], in0=F1[:], in1=zL[:])
            lF = tmp()
            A.activation(lF[:], Ff[:], AF.Ln, bias=1e-30)
            dE = tmp()
            A.activation(dE[:], lF[:], AF.Exp, scale=0.5,
                         accum_out=acc[:, k:k + 1])

            if debug_dump and k == 0:
                for nm, t in [("lin", W1), ("f", WF), ("al", AL), ("be", BE),
                              ("zl", zL), ("ccp", CCp), ("rsqg", rsqG),
                              ("ap", AP), ("cps", CPS), ("dcp", dCp),
                              ("scp", Scp), ("rsqc", rsqC), ("cross", cross),
                              ("cb", cb), ("sb", sb), ("dhps", dHps),
                              ("tt", Tt), ("sch", SCH), ("tc", tC),
                              ("th", tH), ("eg", eg), ("s2d", s2d),
                              ("ff", Ff), ("de", dE)]:
                    w = t.shape[-1]
                    dd = nc.dram_tensor(f"dbg_{nm}", [P, w], t.dtype,
                                        kind="ExternalOutput").ap()
                    nc.sync.dma_start(dd[:], t[:])

        # final: reduce acc cols -> [P,1], DMA out
        accsum = pool.tile([P, 1], F32, tag="accsum", name="accsum")
        V.tensor_reduce(accsum[:], acc[:], mybir.AxisListType.X, ALU.add)
        nc.sync.dma_start(out_d[:], accsum[:])

    nc.compile()
    return nc


def _get_nc():
    if "nc" not in _NC_CACHE:
        _NC_CACHE["nc"] = build_nc()
    return _NC_CACHE["nc"]


def kernel(x: np.ndarray, y: np.ndarray) -> np.ndarray:
    assert x.shape == (32, 3, 512, 512) and y.shape == (32, 3, 512, 512)
    nc = _get_nc()
    shp = (IMGS_PER_CORE, 3, ROWS_PER_IMG, NCHUNK, FCH)
    xs = np.ascontiguousarray(x, dtype=np.float32)
    ys = np.ascontiguousarray(y, dtype=np.float32)
    in_maps = []
    for c in range(NCORE):
        xi = xs[c * IMGS_PER_CORE:(c + 1) * IMGS_PER_CORE].reshape(shp)
        yi = ys[c * IMGS_PER_CORE:(c + 1) * IMGS_PER_CORE].reshape(shp)
        in_maps.append({"x": xi, "y": yi})
    trace = bool(int(os.environ.get("COLOR_TRACE", "0")))
    res = run_bass_kernel_spmd(nc, in_maps, core_ids=list(range(NCORE)),
                               trace=trace)
    _NC_CACHE["last_results"] = res
    total = np.float64(0.0)
    for c in range(NCORE):
        total += np.float64(res.results[c]["out"].sum())
    npix = 32 * 512 * 512
    return np.float32(total / npix / 100.0)
